# revision 8
# baseline (speedup 1.0000x reference)
"""MiniGPT forward (B=8,T=1024,V=8192,D=768,H=12,L=6) on 8 trn2 NeuronCores.

Sharding: pure data-parallel over batch (1 batch row per core, no collectives).
Per-core layout: activations kept feature-major ("xT": [d partitions, t free]),
which makes every matmul transpose-free:
  - q/k produced directly transposed per head-pair: qT/kT [e, t]
  - v produced token-major [t, e] (augmented with a ones column so the
    attention-probability row-sum falls out of the same matmul)
  - scores computed as scoresT [k, q]; softmax sum over k via the ones row,
    normalization applied to the (tiny) oT output instead of the probs
  - causal mask: tile-level skipping + one 128x128 affine_select per diagonal
    subtile applied after exp (fill=0)
  - LN stats via ones-column matmul reductions; normalize with PE-broadcast
    rows; gains/biases fold into one per-partition scalar-engine activation
  - lm head emits logits token-major plus a fused exp row-sum (accum_out) so
    the cross-entropy reduces on host from sum-exp + gathered target logits.
W2 is fed in bf16 (host-cast) to halve its DMA and SBUF footprint.
"""

import numpy as np
import ml_dtypes

import concourse.bass as bass
import concourse.tile as tile
from concourse import bacc, mybir
from concourse import bass_utils
from concourse.masks import make_identity

B, T, V, D, H, L = 8, 1024, 8192, 768, 12, 6
DH = D // H
FF = 4 * D
EPS = 1e-5
SCALE = float(D) ** -0.5
DC = D // 128          # 6 feature chunks
TTN = T // 512         # 2 t-tiles of 512
TCN = T // 128         # 8 t-chunks of 128
FCN = FF // 128        # 24 ff chunks
VTN = V // 512         # 16 vocab tiles
F32 = mybir.dt.float32
BF16 = mybir.dt.bfloat16


def _ln_layer(nc, ps, lnvec, lnscratch, xin, gvec, bvec, xout):
    """LayerNorm over the feature (partition) axis of xin [128, DC, T] -> xout."""
    for tt in range(TTN):
        sl = bass.ts(tt, 512)
        s1 = ps.tile([1, 512], F32, tag="mm", bufs=3, name="ln_s1")
        s2 = ps.tile([1, 512], F32, tag="mm", bufs=3, name="ln_s2")
        for dc in range(DC):
            nc.tensor.matmul(s1, nc._ones_col[:], xin[:, dc, sl],
                             start=(dc == 0), stop=(dc == DC - 1))
        for dc in range(DC):
            sq = lnscratch.tile([128, 512], F32, tag="lnscratch", bufs=3, name="ln_sq")
            nc.scalar.square(sq[:], xin[:, dc, sl])
            nc.tensor.matmul(s2, nc._ones_col[:], sq[:],
                             start=(dc == 0), stop=(dc == DC - 1))
        m = lnvec.tile([1, 512], F32, tag="lnvec", bufs=6, name="ln_m")
        ex2 = lnvec.tile([1, 512], F32, tag="lnvec", bufs=6, name="ln_ex2")
        nc.vector.tensor_scalar_mul(m[:], s1, 1.0 / D)
        nc.vector.tensor_scalar_mul(ex2[:], s2, 1.0 / D)
        var = lnvec.tile([1, 512], F32, tag="lnvec", bufs=6, name="ln_var")
        nc.vector.tensor_mul(var[:], m[:], m[:])
        nc.vector.tensor_tensor(out=var[:], in0=ex2[:], in1=var[:],
                                op=mybir.AluOpType.subtract)
        std = lnvec.tile([1, 512], F32, tag="lnvec", bufs=6, name="ln_std")
        nc.scalar.activation(std[:], var[:], mybir.ActivationFunctionType.Sqrt,
                             bias=nc._eps_tile[:], scale=1.0)
        r = lnvec.tile([1, 512], F32, tag="lnvec", bufs=6, name="ln_r")
        nc.vector.reciprocal(r[:], std[:])
        mr = lnvec.tile([1, 512], F32, tag="lnvec", bufs=6, name="ln_mr")
        nc.vector.tensor_mul(mr[:], m[:], r[:])
        rb = ps.tile([128, 512], F32, tag="bc", bufs=2, name="ln_rb")
        nc.tensor.matmul(rb, nc._ones_row[:, :128], r[:], start=True, stop=True)
        mrb = ps.tile([128, 512], F32, tag="bc", bufs=2, name="ln_mrb")
        nc.tensor.matmul(mrb, nc._ones_row[:, :128], mr[:], start=True, stop=True)
        for dc in range(DC):
            u = lnscratch.tile([128, 512], F32, tag="lnscratch", bufs=3, name="ln_u")
            nc.vector.tensor_mul(u[:], xin[:, dc, sl], rb)
            nc.vector.tensor_tensor(out=u[:], in0=u[:], in1=mrb,
                                    op=mybir.AluOpType.subtract)
            nc.scalar.activation(xout[:, dc, sl], u[:],
                                 mybir.ActivationFunctionType.Identity,
                                 bias=bvec[:, dc:dc + 1], scale=gvec[:, dc:dc + 1])


def _body(ctx, tc, aps):
    nc = tc.nc
    pers = ctx.enter_context(tc.tile_pool(name="pers", bufs=1))
    ps = ctx.enter_context(tc.tile_pool(name="ps", bufs=1, space="PSUM"))
    lnvec = ctx.enter_context(tc.tile_pool(name="lnvec", bufs=1))
    lnscratch = ctx.enter_context(tc.tile_pool(name="lnscratch", bufs=1))

    identity = pers.tile([128, 128], F32, name="identity")
    make_identity(nc, identity[:])
    ones_col = pers.tile([128, 1], F32, name="ones_col")
    nc.vector.memset(ones_col[:], 1.0)
    ones_row = pers.tile([1, 512], F32, name="ones_row")
    nc.vector.memset(ones_row[:], 1.0)
    eps_tile = pers.tile([1, 1], F32, name="eps_tile")
    nc.vector.memset(eps_tile[:], EPS)
    nc._ones_col = ones_col
    nc._ones_row = ones_row
    nc._eps_tile = eps_tile

    xT = pers.tile([128, DC, T], F32, name="xT")
    hbuf = pers.tile([128, DC, T], F32, name="hbuf")

    idx_sb = pers.tile([128, TCN], mybir.dt.int32, name="idx_sb")
    nc.sync.dma_start(idx_sb[:], aps["idx"].rearrange("(c p) -> p c", p=128))

    # ---- embedding gather + transpose into feature-major xT ----
    with tc.tile_pool(name="embed", bufs=3) as emb:
        for tcn in range(TCN):
            x0 = emb.tile([128, D], F32, tag="x0", bufs=3, name="emb_x0")
            nc.gpsimd.indirect_dma_start(
                out=x0[:], out_offset=None, in_=aps["tok_emb"],
                in_offset=bass.IndirectOffsetOnAxis(ap=idx_sb[:, tcn:tcn + 1], axis=0))
            pos = emb.tile([128, D], F32, tag="pos", bufs=3, name="emb_pos")
            nc.sync.dma_start(pos[:], aps["pos_emb"][bass.ts(tcn, 128), :])
            nc.vector.tensor_add(x0[:], x0[:], pos[:])
            for dc in range(DC):
                tp = ps.tile([128, 128], F32, tag="mm", bufs=3, name="emb_tp")
                nc.tensor.transpose(tp, x0[:, bass.ts(dc, 128)], identity[:])
                nc.vector.tensor_copy(xT[:, dc, bass.ts(tcn, 128)], tp)

    lvec = ctx.enter_context(tc.tile_pool(name="lvec", bufs=2))

    with tc.tile_pool(name="lay", bufs=1) as lay:
        for l in range(L):
            g1 = lvec.tile([128, DC], F32, tag="g1", name="g1")
            nc.sync.dma_start(g1[:], aps["ln1_g"][l].rearrange("(c p) -> p c", p=128))
            b1l = lvec.tile([128, DC], F32, tag="b1l", name="b1l")
            nc.sync.dma_start(b1l[:], aps["ln1_b"][l].rearrange("(c p) -> p c", p=128))
            g2 = lvec.tile([128, DC], F32, tag="g2", name="g2")
            nc.sync.dma_start(g2[:], aps["ln2_g"][l].rearrange("(c p) -> p c", p=128))
            b2l = lvec.tile([128, DC], F32, tag="b2l", name="b2l")
            nc.sync.dma_start(b2l[:], aps["ln2_b"][l].rearrange("(c p) -> p c", p=128))
            bpj = lvec.tile([1, D], F32, tag="bpj", name="bpj")
            nc.sync.dma_start(bpj[:], aps["bproj"][l:l + 1, :])
            b1f = lvec.tile([128, FCN], F32, tag="b1f", name="b1f")
            nc.sync.dma_start(b1f[:], aps["b1"][l].rearrange("(c p) -> p c", p=128))
            b2f = lvec.tile([1, D], F32, tag="b2f", name="b2f")
            nc.sync.dma_start(b2f[:], aps["b2"][l:l + 1, :])

            # ---- LN1: xT -> hbuf ----
            _ln_layer(nc, ps, lnvec, lnscratch, xT, g1, b1l, hbuf)

            # ---- qT / kT (feature-major, per head-pair chunks) ----
            qT = lay.tile([128, DC, T], F32, tag="qT", name="qT")
            kT = lay.tile([128, DC, T], F32, tag="kT", name="kT")
            for c in range(DC):
                wq = lay.tile([128, DC, 2, DH], F32, tag="wstream", bufs=2, name="wq")
                wk = lay.tile([128, DC, 2, DH], F32, tag="wstream", bufs=2, name="wk")
                for hh in range(2):
                    nc.sync.dma_start(
                        wq[:, :, hh, :], aps["Wq"][l, 2 * c + hh]
                        .rearrange("(dc p) e -> p dc e", p=128))
                    nc.sync.dma_start(
                        wk[:, :, hh, :], aps["Wk"][l, 2 * c + hh]
                        .rearrange("(dc p) e -> p dc e", p=128))
                for tt in range(TTN):
                    sl = bass.ts(tt, 512)
                    qps = ps.tile([128, 512], F32, tag="mm", bufs=3, name="q_ps")
                    for dc in range(DC):
                        nc.tensor.matmul(qps, wq[:, dc].rearrange("p h e -> p (h e)"),
                                         hbuf[:, dc, sl],
                                         start=(dc == 0), stop=(dc == DC - 1))
                    nc.scalar.copy(qT[:, c, sl], qps)
                    kps = ps.tile([128, 512], F32, tag="mm", bufs=3, name="k_ps")
                    for dc in range(DC):
                        nc.tensor.matmul(kps, wk[:, dc].rearrange("p h e -> p (h e)"),
                                         hbuf[:, dc, sl],
                                         start=(dc == 0), stop=(dc == DC - 1))
                    nc.scalar.copy(kT[:, c, sl], kps)

            # ---- v (token-major, all heads, augmented with ones column) ----
            v_aug = lay.tile([128, TCN, H, DH + 1], F32, tag="big", name="v_aug")
            nc.vector.memset(v_aug[:, :, :, DH:DH + 1], 1.0)
            wv0 = lay.tile([128, DC, 8, DH], F32, tag="wstream", bufs=2, name="wv0")
            wv1 = lay.tile([128, DC, 4, DH], F32, tag="wstream", bufs=2, name="wv1")
            for hh in range(8):
                nc.sync.dma_start(wv0[:, :, hh, :], aps["Wv"][l, hh]
                                  .rearrange("(dc p) e -> p dc e", p=128))
            for hh in range(4):
                nc.sync.dma_start(wv1[:, :, hh, :], aps["Wv"][l, 8 + hh]
                                  .rearrange("(dc p) e -> p dc e", p=128))
            for tcn in range(TCN):
                v0 = ps.tile([128, 512], F32, tag="mm", bufs=3, name="v0_ps")
                for dc in range(DC):
                    nc.tensor.matmul(v0, hbuf[:, dc, bass.ts(tcn, 128)],
                                     wv0[:, dc].rearrange("p h e -> p (h e)"),
                                     start=(dc == 0), stop=(dc == DC - 1))
                nc.vector.tensor_copy(
                    v_aug[:, tcn, 0:8, 0:DH],
                    v0.rearrange("p (h e) -> p h e", e=DH))
                v1 = ps.tile([128, 256], F32, tag="mm", bufs=3, name="v1_ps")
                for dc in range(DC):
                    nc.tensor.matmul(v1, hbuf[:, dc, bass.ts(tcn, 128)],
                                     wv1[:, dc].rearrange("p h e -> p (h e)"),
                                     start=(dc == 0), stop=(dc == DC - 1))
                nc.vector.tensor_copy(
                    v_aug[:, tcn, 8:12, 0:DH],
                    v1.rearrange("p (h e) -> p h e", e=DH))

            # ---- attention (scoresT layout [k, q]) -> hbuf (= concat-head oT) ----
            for h in range(H):
                c, po = h // 2, 64 * (h % 2)
                for qt in range(TTN):
                    nk = 4 * (qt + 1)
                    ot = ps.tile([65, 512], F32, tag="acc", bufs=3, name="ot_ps")
                    for ki in range(nk):
                        sc = ps.tile([128, 512], F32, tag="mm", bufs=3, name="sc_ps")
                        nc.tensor.matmul(
                            sc, kT[po:po + 64, c, bass.ts(ki, 128)],
                            qT[po:po + 64, c, bass.ts(qt, 512)],
                            start=True, stop=True)
                        ex = lay.tile([128, 512], F32, tag="expT", bufs=4, name="ex")
                        r = ki - 4 * qt
                        if r < 0:
                            nc.scalar.activation(
                                ex[:], sc, mybir.ActivationFunctionType.Exp,
                                scale=SCALE)
                        else:
                            if r > 0:
                                nc.vector.memset(ex[:, :128 * r], 0.0)
                            nc.scalar.activation(
                                ex[:, 128 * r:], sc[:, 128 * r:],
                                mybir.ActivationFunctionType.Exp, scale=SCALE)
                            nc.gpsimd.affine_select(
                                out=ex[:, 128 * r:128 * (r + 1)],
                                in_=ex[:, 128 * r:128 * (r + 1)],
                                compare_op=mybir.AluOpType.is_ge,
                                fill=0.0, base=0, channel_multiplier=-1,
                                pattern=[[1, 128]])
                        nc.tensor.matmul(
                            ot, v_aug[:, ki, h, :], ex[:],
                            start=(ki == 0), stop=(ki == nk - 1))
                    rr = lnvec.tile([1, 512], F32, tag="rr", bufs=2, name="rr")
                    nc.vector.reciprocal(rr[:], ot[64:65, :])
                    rbc = ps.tile([64, 512], F32, tag="bc", bufs=2, name="rbc")
                    nc.tensor.matmul(rbc, nc._ones_row[:, :64], rr[:],
                                     start=True, stop=True)
                    rbs = lnscratch.tile([64, 512], F32, tag="rbs", bufs=2, name="rbs")
                    nc.vector.tensor_copy(rbs[:], rbc)
                    nc.vector.tensor_mul(
                        hbuf[po:po + 64, c, bass.ts(qt, 512)], ot[0:64, :], rbs[:])

            # ---- proj + residual into xT ----
            for pc in range(DC):
                wp = lay.tile([128, DC, 128], F32, tag="wstream", bufs=2, name="wp")
                nc.sync.dma_start(
                    wp[:], aps["Wproj"][l].rearrange("(dc p) n -> p dc n", p=128)
                    [:, :, bass.ts(pc, 128)])
                for tt in range(TTN):
                    sl = bass.ts(tt, 512)
                    pj = ps.tile([128, 512], F32, tag="mm", bufs=3, name="pj_ps")
                    for dc in range(DC):
                        nc.tensor.matmul(pj, wp[:, dc], hbuf[:, dc, sl],
                                         start=(dc == 0), stop=False)
                    nc.tensor.matmul(pj, bpj[:, bass.ts(pc, 128)], ones_row[:],
                                     start=False, stop=True)
                    nc.vector.tensor_add(xT[:, pc, sl], xT[:, pc, sl], pj)

            # ---- LN2: xT -> hbuf ----
            _ln_layer(nc, ps, lnvec, lnscratch, xT, g2, b2l, hbuf)

            # ---- FFN + residual into xT ----
            for tt in range(TTN):
                sl = bass.ts(tt, 512)
                hT = lay.tile([128, FCN, 512], BF16, tag="big", name="hT")
                for fc in range(FCN):
                    w1 = lay.tile([128, DC, 128], F32, tag="wstream", bufs=2,
                                  name="w1")
                    nc.sync.dma_start(
                        w1[:], aps["W1"][l].rearrange("(dc p) n -> p dc n", p=128)
                        [:, :, bass.ts(fc, 128)])
                    h1 = ps.tile([128, 512], F32, tag="mm", bufs=3, name="h1_ps")
                    for dc in range(DC):
                        nc.tensor.matmul(h1, w1[:, dc], hbuf[:, dc, sl],
                                         start=(dc == 0), stop=(dc == DC - 1))
                    nc.scalar.activation(hT[:, fc, :], h1,
                                         mybir.ActivationFunctionType.Relu,
                                         bias=b1f[:, fc:fc + 1], scale=1.0)
                for dc2 in range(DC):
                    w2 = lay.tile([128, FCN, 128], BF16, tag="wstream", bufs=2,
                                  name="w2")
                    nc.sync.dma_start(
                        w2[:], aps["W2bf"][l].rearrange("(fc p) n -> p fc n", p=128)
                        [:, :, bass.ts(dc2, 128)])
                    o2 = ps.tile([128, 512], F32, tag="acc", bufs=3, name="o2_ps")
                    for fc in range(FCN):
                        nc.tensor.matmul(o2, w2[:, fc], hT[:, fc, :],
                                         start=(fc == 0), stop=False)
                    nc.tensor.matmul(o2, b2f[:, bass.ts(dc2, 128)], ones_row[:],
                                     start=False, stop=True)
                    nc.vector.tensor_add(xT[:, dc2, sl], xT[:, dc2, sl], o2)

    # ---- final LN ----
    gf = lvec.tile([128, DC], F32, tag="g1", name="gf")
    nc.sync.dma_start(gf[:], aps["lnf_g"].rearrange("(c p) -> p c", p=128))
    bf = lvec.tile([128, DC], F32, tag="b1l", name="bf")
    nc.sync.dma_start(bf[:], aps["lnf_b"].rearrange("(c p) -> p c", p=128))
    _ln_layer(nc, ps, lnvec, lnscratch, xT, gf, bf, hbuf)

    # ---- lm head + fused sum-exp ----
    with tc.tile_pool(name="lm", bufs=1) as lm:
        blm = lm.tile([128, V], F32, name="blm")
        nc.sync.dma_start(
            blm[:], bass.AP(tensor=aps["b_lm"].tensor, offset=aps["b_lm"].offset,
                            ap=[[0, 128]] + list(aps["b_lm"].ap)))
        spart = lm.tile([128, TCN, VTN], F32, name="spart")
        for vt in range(VTN):
            wlm = lm.tile([128, DC, 512], F32, tag="wlm", bufs=2, name="wlm")
            nc.sync.dma_start(
                wlm[:], aps["W_lm"].rearrange("(dc p) n -> p dc n", p=128)
                [:, :, bass.ts(vt, 512)])
            for tcn in range(TCN):
                lg = ps.tile([128, 512], F32, tag="mm", bufs=3, name="lg_ps")
                for dc in range(DC):
                    nc.tensor.matmul(lg, hbuf[:, dc, bass.ts(tcn, 128)], wlm[:, dc],
                                     start=(dc == 0), stop=(dc == DC - 1))
                lgs = lm.tile([128, 512], F32, tag="lgs", bufs=4, name="lgs")
                nc.vector.tensor_add(lgs[:], lg, blm[:, bass.ts(vt, 512)])
                nc.sync.dma_start(
                    aps["logits"][bass.ts(tcn, 128), bass.ts(vt, 512)], lgs[:])
                exps = lm.tile([128, 512], F32, tag="exps", bufs=3, name="exps")
                nc.scalar.activation(exps[:], lgs[:],
                                     mybir.ActivationFunctionType.Exp,
                                     accum_out=spart[:, tcn, vt:vt + 1])
        ssum = lm.tile([128, TCN], F32, name="ssum")
        for tcn in range(TCN):
            nc.vector.reduce_sum(out=ssum[:, tcn:tcn + 1], in_=spart[:, tcn, :],
                                 axis=mybir.AxisListType.X)
        nc.sync.dma_start(aps["sumexp"], ssum[:])


_NC_CACHE = None


def _build():
    global _NC_CACHE
    if _NC_CACHE is not None:
        return _NC_CACHE
    from contextlib import ExitStack

    nc = bacc.Bacc("TRN2", target_bir_lowering=False, debug=False, num_devices=8)
    aps = {}
    ins = {
        "idx": ([T], mybir.dt.int32),
        "tok_emb": ([V, D], F32),
        "pos_emb": ([T, D], F32),
        "ln1_g": ([L, D], F32), "ln1_b": ([L, D], F32),
        "Wq": ([L, H, D, DH], F32), "Wk": ([L, H, D, DH], F32),
        "Wv": ([L, H, D, DH], F32),
        "Wproj": ([L, D, D], F32), "bproj": ([L, D], F32),
        "ln2_g": ([L, D], F32), "ln2_b": ([L, D], F32),
        "W1": ([L, D, FF], F32), "b1": ([L, FF], F32),
        "W2bf": ([L, FF, D], BF16), "b2": ([L, D], F32),
        "lnf_g": ([D], F32), "lnf_b": ([D], F32),
        "W_lm": ([D, V], F32), "b_lm": ([V], F32),
    }
    for name, (shape, dt) in ins.items():
        aps[name] = nc.dram_tensor(name, shape, dt, kind="ExternalInput").ap()
    aps["logits"] = nc.dram_tensor("logits", [T, V], F32, kind="ExternalOutput").ap()
    aps["sumexp"] = nc.dram_tensor("sumexp", [128, TCN], F32,
                                   kind="ExternalOutput").ap()

    with tile.TileContext(nc) as tc:
        with ExitStack() as ctx:
            _body(ctx, tc, aps)
    nc.compile()
    _NC_CACHE = nc
    return nc


def kernel(**inputs):
    nc = _build()
    f32 = lambda k: np.ascontiguousarray(np.asarray(inputs[k], dtype=np.float32))
    idx = np.ascontiguousarray(np.asarray(inputs["idx"], dtype=np.int32))
    targets = np.asarray(inputs["targets"])
    shared = {k: f32(k) for k in
              ["tok_emb", "pos_emb", "ln1_g", "ln1_b", "Wq", "Wk", "Wv", "Wproj",
               "bproj", "ln2_g", "ln2_b", "W1", "b1", "b2", "lnf_g", "lnf_b",
               "W_lm", "b_lm"]}
    shared["W2bf"] = np.ascontiguousarray(
        np.asarray(inputs["W2"], dtype=np.float32).astype(ml_dtypes.bfloat16))
    in_maps = [dict(shared, idx=idx[b]) for b in range(B)]
    res = bass_utils.run_bass_kernel_spmd(nc, in_maps, core_ids=list(range(B)))

    logits = np.stack([res.results[b]["logits"] for b in range(B)])  # [B, T, V]
    # sumexp comes back [128, TCN] with token t = tc*128 + p at [p, tc]
    s = np.stack([res.results[b]["sumexp"].T.reshape(T) for b in range(B)])  # [B,T]
    tgt = np.take_along_axis(
        logits.astype(np.float64), targets[..., None].astype(np.int64), axis=-1
    )[..., 0]
    loss = np.float32(np.mean(np.log(s.astype(np.float64)) - tgt))
    return logits, loss


# revision 15
# speedup vs baseline: 1.6676x; 1.6676x over previous
"""MiniGPT forward (B=8,T=1024,V=8192,D=768,H=12,L=6) on 8 trn2 NeuronCores.

Sharding: pure data-parallel over batch (1 batch row per core, no collectives).
Per-core layout: activations kept feature-major ("xT": [d partitions, t free]),
which makes every matmul transpose-free:
  - q/k produced directly transposed per head-pair: qT/kT [e, t]
  - v produced token-major [t, e] (augmented with a ones column so the
    attention-probability row-sum falls out of the same matmul)
  - scores computed as scoresT [k, q]; softmax sum over k via the ones row,
    normalization applied to the (tiny) oT output instead of the probs
  - causal mask: tile-level skipping + one 128x128 affine_select per diagonal
    subtile applied after exp (fill=0)
  - LN stats via ones-column matmul reductions; normalize with PE-broadcast
    rows; gains/biases fold into one per-partition scalar-engine activation
  - lm head emits logits token-major plus a fused exp row-sum (accum_out) so
    the cross-entropy reduces on host from sum-exp + gathered target logits.
All matmul operands are float32r (TF32-style) for full PE rate at fp32 I/O.
"""

import numpy as np

import concourse.bass as bass
import concourse.tile as tile
from concourse import bacc, mybir
from concourse import bass_utils
from concourse.masks import make_identity

B, T, V, D, H, L = 8, 1024, 8192, 768, 12, 6
DH = D // H
FF = 4 * D
EPS = 1e-5
SCALE = float(D) ** -0.5
DC = D // 128          # 6 feature chunks
TTN = T // 512         # 2 t-tiles of 512
TCN = T // 128         # 8 t-chunks of 128
FCN = FF // 128        # 24 ff chunks
FH = FCN // 2          # ff half-block
VTN = V // 512         # 16 vocab tiles
F32 = mybir.dt.float32
F32R = mybir.dt.float32r
I32 = mybir.dt.int32


def _ln_layer(nc, P, xin, gvec, bvec, xout):
    """LayerNorm over the feature (partition) axis of xin [128, DC, T] -> xout."""
    ps, lnvec, lnscratch = P["ps"], P["lnvec"], P["lnscratch"]
    for tt in range(TTN):
        sl = bass.ts(tt, 512)
        s1 = ps.tile([1, 512], F32, tag="mm", bufs=3, name="ln_s1")
        s2 = ps.tile([1, 512], F32, tag="mm", bufs=3, name="ln_s2")
        for dc in range(DC):
            nc.tensor.matmul(s1, nc._ones_col[:], xin[:, dc, sl],
                             start=(dc == 0), stop=(dc == DC - 1))
        for dc in range(DC):
            sq = lnscratch.tile([128, 512], F32R, tag="lnscratch", bufs=3,
                                name="ln_sq")
            nc.scalar.square(sq[:], xin[:, dc, sl])
            nc.tensor.matmul(s2, nc._ones_col[:], sq[:],
                             start=(dc == 0), stop=(dc == DC - 1))
        m = lnvec.tile([1, 512], F32, tag="lnvec", bufs=6, name="ln_m")
        ex2 = lnvec.tile([1, 512], F32, tag="lnvec", bufs=6, name="ln_ex2")
        nc.vector.tensor_scalar_mul(m[:], s1, 1.0 / D)
        nc.vector.tensor_scalar_mul(ex2[:], s2, 1.0 / D)
        var = lnvec.tile([1, 512], F32, tag="lnvec", bufs=6, name="ln_var")
        nc.vector.tensor_mul(var[:], m[:], m[:])
        nc.vector.tensor_tensor(out=var[:], in0=ex2[:], in1=var[:],
                                op=mybir.AluOpType.subtract)
        std = lnvec.tile([1, 512], F32, tag="lnvec", bufs=6, name="ln_std")
        nc.scalar.activation(std[:], var[:], mybir.ActivationFunctionType.Sqrt,
                             bias=nc._eps_tile[:], scale=1.0)
        r = lnvec.tile([1, 512], F32R, tag="lnvec", bufs=6, name="ln_r")
        nc.vector.reciprocal(r[:], std[:])
        mr = lnvec.tile([1, 512], F32R, tag="lnvec", bufs=6, name="ln_mr")
        nc.vector.tensor_mul(mr[:], m[:], r[:])
        rb = ps.tile([128, 512], F32, tag="bc", bufs=2, name="ln_rb")
        nc.tensor.matmul(rb, nc._ones_row[:, :128], r[:], start=True, stop=True)
        mrb = ps.tile([128, 512], F32, tag="bc", bufs=2, name="ln_mrb")
        nc.tensor.matmul(mrb, nc._ones_row[:, :128], mr[:], start=True, stop=True)
        for dc in range(DC):
            u = lnscratch.tile([128, 512], F32, tag="lnscratch", bufs=3,
                               name="ln_u")
            nc.vector.tensor_mul(u[:], xin[:, dc, sl], rb)
            nc.vector.tensor_tensor(out=u[:], in0=u[:], in1=mrb,
                                    op=mybir.AluOpType.subtract)
            nc.scalar.activation(xout[:, dc, sl], u[:],
                                 mybir.ActivationFunctionType.Identity,
                                 bias=bvec[:, dc:dc + 1], scale=gvec[:, dc:dc + 1])


def _forward(nc, P, aps, xT, hbuf, idx_sb, identity):
    """One full forward pass. Pools in P, persistent tiles passed in."""
    ps, lay, lvec = P["ps"], P["lay"], P["lvec"]
    lnvec, lnscratch = P["lnvec"], P["lnscratch"]
    ones_row = nc._ones_row

    # ---- embedding gather + transpose into feature-major xT ----
    for tcn in range(TCN):
        x0 = lnscratch.tile([128, D], F32, tag="lnscratch", bufs=3, name="emb_x0")
        nc.gpsimd.indirect_dma_start(
            out=x0[:], out_offset=None, in_=aps["tok_emb"],
            in_offset=bass.IndirectOffsetOnAxis(ap=idx_sb[:, tcn:tcn + 1], axis=0))
        pos = lnscratch.tile([128, D], F32, tag="lnscratch", bufs=3, name="emb_pos")
        nc.sync.dma_start(pos[:], aps["pos_emb"][bass.ts(tcn, 128), :])
        nc.vector.tensor_add(x0[:], x0[:], pos[:])
        for dc in range(DC):
            tp = ps.tile([128, 128], F32, tag="mm", bufs=3, name="emb_tp")
            nc.tensor.transpose(tp, x0[:, bass.ts(dc, 128)], identity[:])
            nc.vector.tensor_copy(xT[:, dc, bass.ts(tcn, 128)], tp)

    for l in range(L):
        g1 = lvec.tile([128, DC], F32, tag="g1", name="g1")
        nc.sync.dma_start(g1[:], aps["ln1_g"][l].rearrange("(c p) -> p c", p=128))
        b1l = lvec.tile([128, DC], F32, tag="b1l", name="b1l")
        nc.sync.dma_start(b1l[:], aps["ln1_b"][l].rearrange("(c p) -> p c", p=128))
        g2 = lvec.tile([128, DC], F32, tag="g2", name="g2")
        nc.sync.dma_start(g2[:], aps["ln2_g"][l].rearrange("(c p) -> p c", p=128))
        b2l = lvec.tile([128, DC], F32, tag="b2l", name="b2l")
        nc.sync.dma_start(b2l[:], aps["ln2_b"][l].rearrange("(c p) -> p c", p=128))
        bpj = lvec.tile([1, D], F32R, tag="bpj", name="bpj")
        nc.sync.dma_start(bpj[:], aps["bproj"][l:l + 1, :])
        b1f = lvec.tile([128, FCN], F32, tag="b1f", name="b1f")
        nc.sync.dma_start(b1f[:], aps["b1"][l].rearrange("(c p) -> p c", p=128))
        b2f = lvec.tile([1, D], F32R, tag="b2f", name="b2f")
        nc.sync.dma_start(b2f[:], aps["b2"][l:l + 1, :])

        # ---- LN1: xT -> hbuf ----
        _ln_layer(nc, P, xT, g1, b1l, hbuf)

        # ---- qT / kT (feature-major, per head-pair chunks) ----
        qT = lay.tile([128, DC, T], F32R, tag="qT", name="qT")
        kT = lay.tile([128, DC, T], F32R, tag="kT", name="kT")
        for c in range(DC):
            wq = lay.tile([128, DC, 2, DH], F32R, tag="wstream", bufs=2, name="wq")
            wk = lay.tile([128, DC, 2, DH], F32R, tag="wstream", bufs=2, name="wk")
            for hh in range(2):
                nc.sync.dma_start(
                    wq[:, :, hh, :], aps["Wq"][l, 2 * c + hh]
                    .rearrange("(dc p) e -> p dc e", p=128))
                nc.sync.dma_start(
                    wk[:, :, hh, :], aps["Wk"][l, 2 * c + hh]
                    .rearrange("(dc p) e -> p dc e", p=128))
            for tt in range(TTN):
                sl = bass.ts(tt, 512)
                qps = ps.tile([128, 512], F32, tag="mm", bufs=3, name="q_ps")
                for dc in range(DC):
                    nc.tensor.matmul(qps, wq[:, dc].rearrange("p h e -> p (h e)"),
                                     hbuf[:, dc, sl],
                                     start=(dc == 0), stop=(dc == DC - 1))
                nc.scalar.copy(qT[:, c, sl], qps)
                kps = ps.tile([128, 512], F32, tag="mm", bufs=3, name="k_ps")
                for dc in range(DC):
                    nc.tensor.matmul(kps, wk[:, dc].rearrange("p h e -> p (h e)"),
                                     hbuf[:, dc, sl],
                                     start=(dc == 0), stop=(dc == DC - 1))
                nc.scalar.copy(kT[:, c, sl], kps)

        # ---- v (token-major, all heads, augmented with ones column) ----
        v_aug = lay.tile([128, TCN, H, DH + 1], F32R, tag="big", name="v_aug")
        nc.vector.memset(v_aug[:, :, :, DH:DH + 1].bitcast(F32), 1.0)
        wv0 = lay.tile([128, DC, 8, DH], F32R, tag="wstream", bufs=2, name="wv0")
        wv1 = lay.tile([128, DC, 4, DH], F32R, tag="wstream", bufs=2, name="wv1")
        for hh in range(8):
            nc.sync.dma_start(wv0[:, :, hh, :], aps["Wv"][l, hh]
                              .rearrange("(dc p) e -> p dc e", p=128))
        for hh in range(4):
            nc.sync.dma_start(wv1[:, :, hh, :], aps["Wv"][l, 8 + hh]
                              .rearrange("(dc p) e -> p dc e", p=128))
        for tcn in range(TCN):
            v0 = ps.tile([128, 512], F32, tag="mm", bufs=3, name="v0_ps")
            for dc in range(DC):
                nc.tensor.matmul(v0, hbuf[:, dc, bass.ts(tcn, 128)],
                                 wv0[:, dc].rearrange("p h e -> p (h e)"),
                                 start=(dc == 0), stop=(dc == DC - 1))
            nc.vector.tensor_copy(
                v_aug[:, tcn, 0:8, 0:DH], v0.rearrange("p (h e) -> p h e", e=DH))
            v1 = ps.tile([128, 256], F32, tag="mm", bufs=3, name="v1_ps")
            for dc in range(DC):
                nc.tensor.matmul(v1, hbuf[:, dc, bass.ts(tcn, 128)],
                                 wv1[:, dc].rearrange("p h e -> p (h e)"),
                                 start=(dc == 0), stop=(dc == DC - 1))
            nc.vector.tensor_copy(
                v_aug[:, tcn, 8:12, 0:DH], v1.rearrange("p (h e) -> p h e", e=DH))

        # ---- attention (scoresT layout [k, q]) -> hbuf (= concat-head oT) ----
        for h in range(H):
            c, po = h // 2, 64 * (h % 2)
            for qt in range(TTN):
                nk = 4 * (qt + 1)
                ot = ps.tile([65, 512], F32, tag="acc", bufs=3, name="ot_ps")
                for ki in range(nk):
                    sc = ps.tile([128, 512], F32, tag="mm", bufs=3, name="sc_ps")
                    nc.tensor.matmul(
                        sc, kT[po:po + 64, c, bass.ts(ki, 128)],
                        qT[po:po + 64, c, bass.ts(qt, 512)],
                        start=True, stop=True)
                    ex = lay.tile([128, 512], F32R, tag="expT", bufs=3, name="ex")
                    rdiag = ki - 4 * qt
                    if rdiag < 0:
                        nc.scalar.activation(
                            ex[:], sc, mybir.ActivationFunctionType.Exp,
                            scale=SCALE)
                    else:
                        if rdiag > 0:
                            nc.vector.memset(ex[:, :128 * rdiag].bitcast(F32), 0.0)
                        nc.scalar.activation(
                            ex[:, 128 * rdiag:], sc[:, 128 * rdiag:],
                            mybir.ActivationFunctionType.Exp, scale=SCALE)
                        nc.gpsimd.affine_select(
                            out=ex[:, 128 * rdiag:128 * (rdiag + 1)],
                            in_=ex[:, 128 * rdiag:128 * (rdiag + 1)],
                            compare_op=mybir.AluOpType.is_ge,
                            fill=0.0, base=0, channel_multiplier=-1,
                            pattern=[[1, 128]])
                    nc.tensor.matmul(
                        ot, v_aug[:, ki, h, :], ex[:],
                        start=(ki == 0), stop=(ki == nk - 1))
                rr = lnvec.tile([1, 512], F32R, tag="rr", bufs=2, name="rr")
                nc.vector.reciprocal(rr[:], ot[64:65, :])
                rbc = ps.tile([64, 512], F32, tag="bc", bufs=2, name="rbc")
                nc.tensor.matmul(rbc, nc._ones_row[:, :64], rr[:],
                                 start=True, stop=True)
                rbs = lnscratch.tile([64, 512], F32, tag="rbs", bufs=2, name="rbs")
                nc.vector.tensor_copy(rbs[:], rbc)
                nc.vector.tensor_mul(
                    hbuf[po:po + 64, c, bass.ts(qt, 512)], ot[0:64, :], rbs[:])

        # ---- proj + residual into xT ----
        for pc in range(DC):
            wp = lay.tile([128, DC, 128], F32R, tag="wstream", bufs=2, name="wp")
            nc.sync.dma_start(
                wp[:], aps["Wproj"][l].rearrange("(dc p) n -> p dc n", p=128)
                [:, :, bass.ts(pc, 128)])
            for tt in range(TTN):
                sl = bass.ts(tt, 512)
                pj = ps.tile([128, 512], F32, tag="mm", bufs=3, name="pj_ps")
                for dc in range(DC):
                    nc.tensor.matmul(pj, wp[:, dc], hbuf[:, dc, sl],
                                     start=(dc == 0), stop=False)
                nc.tensor.matmul(pj, bpj[:, bass.ts(pc, 128)], ones_row[:],
                                 start=False, stop=True)
                nc.vector.tensor_add(xT[:, pc, sl], xT[:, pc, sl], pj)

        # ---- LN2: xT -> hbuf ----
        _ln_layer(nc, P, xT, g2, b2l, hbuf)

        # ---- FFN + residual into xT (half-ff blocking) ----
        for tt in range(TTN):
            sl = bass.ts(tt, 512)
            o2sb = lay.tile([128, DC, 512], F32, tag="o2sb", name="o2sb")
            for half in range(2):
                hT = lay.tile([128, FH, 512], F32R, tag="big", name="hT")
                for fcl in range(FH):
                    fc = half * FH + fcl
                    w1 = lay.tile([128, DC, 128], F32R, tag="wstream", bufs=2,
                                  name="w1")
                    nc.sync.dma_start(
                        w1[:], aps["W1"][l].rearrange("(dc p) n -> p dc n", p=128)
                        [:, :, bass.ts(fc, 128)])
                    h1 = ps.tile([128, 512], F32, tag="mm", bufs=3, name="h1_ps")
                    for dc in range(DC):
                        nc.tensor.matmul(h1, w1[:, dc], hbuf[:, dc, sl],
                                         start=(dc == 0), stop=(dc == DC - 1))
                    nc.scalar.activation(hT[:, fcl, :], h1,
                                         mybir.ActivationFunctionType.Relu,
                                         bias=b1f[:, fc:fc + 1], scale=1.0)
                for dc2 in range(DC):
                    w2 = lay.tile([128, FH, 128], F32R, tag="wstream", bufs=2,
                                  name="w2")
                    nc.sync.dma_start(
                        w2[:], aps["W2"][l].rearrange("(fc p) n -> p fc n", p=128)
                        [:, bass.ds(half * FH, FH), bass.ts(dc2, 128)])
                    o2 = ps.tile([128, 512], F32, tag="acc", bufs=3, name="o2_ps")
                    for fcl in range(FH):
                        nc.tensor.matmul(o2, w2[:, fcl], hT[:, fcl, :],
                                         start=(fcl == 0),
                                         stop=(half == 0 and fcl == FH - 1))
                    if half == 0:
                        nc.vector.tensor_copy(o2sb[:, dc2, :], o2)
                    else:
                        nc.tensor.matmul(o2, b2f[:, bass.ts(dc2, 128)],
                                         ones_row[:], start=False, stop=True)
                        nc.vector.tensor_add(xT[:, dc2, sl], xT[:, dc2, sl], o2)
                        nc.vector.tensor_add(xT[:, dc2, sl], xT[:, dc2, sl],
                                             o2sb[:, dc2, :])

    # ---- final LN ----
    gf = lvec.tile([128, DC], F32, tag="g1", name="gf")
    nc.sync.dma_start(gf[:], aps["lnf_g"].rearrange("(c p) -> p c", p=128))
    bf = lvec.tile([128, DC], F32, tag="b1l", name="bf")
    nc.sync.dma_start(bf[:], aps["lnf_b"].rearrange("(c p) -> p c", p=128))
    _ln_layer(nc, P, xT, gf, bf, hbuf)

    # ---- lm head + fused sum-exp ----
    spart = P["spart"]
    for vt in range(VTN):
        wlm = lay.tile([128, DC, 512], F32R, tag="wstream", bufs=2, name="wlm")
        nc.sync.dma_start(
            wlm[:], aps["W_lm"].rearrange("(dc p) n -> p dc n", p=128)
            [:, :, bass.ts(vt, 512)])
        blmv = lnvec.tile([1, 512], F32R, tag="rr", bufs=2, name="blmv")
        nc.sync.dma_start(blmv[:], aps["b_lm"].rearrange("(a v) -> a v", a=1)
                          [:, bass.ts(vt, 512)])
        for tcn in range(TCN):
            lg = ps.tile([128, 512], F32, tag="mm", bufs=3, name="lg_ps")
            for dc in range(DC):
                nc.tensor.matmul(lg, hbuf[:, dc, bass.ts(tcn, 128)], wlm[:, dc],
                                 start=(dc == 0), stop=False)
            nc.tensor.matmul(lg, ones_row[:, :128], blmv[:],
                             start=False, stop=True)
            lgs = lay.tile([128, 512], F32, tag="expT", bufs=3, name="lgs")
            nc.vector.tensor_copy(lgs[:], lg)
            nc.sync.dma_start(
                aps["logits"][bass.ts(tcn, 128), bass.ts(vt, 512)], lgs[:])
            exps = lnscratch.tile([128, 512], F32, tag="lnscratch", bufs=3,
                                  name="exps")
            nc.scalar.activation(exps[:], lgs[:],
                                 mybir.ActivationFunctionType.Exp,
                                 accum_out=spart[:, tcn, vt:vt + 1])
    ssum = P["ssum"]
    for tcn in range(TCN):
        nc.vector.reduce_sum(out=ssum[:, tcn:tcn + 1], in_=spart[:, tcn, :],
                             axis=mybir.AxisListType.X)
    nc.sync.dma_start(aps["sumexp"], ssum[:])


def _body(ctx, tc, aps, reps=1):
    nc = tc.nc
    pers = ctx.enter_context(tc.tile_pool(name="pers", bufs=1))
    P = {
        "ps": ctx.enter_context(tc.tile_pool(name="ps", bufs=1, space="PSUM")),
        "lnvec": ctx.enter_context(tc.tile_pool(name="lnvec", bufs=1)),
        "lnscratch": ctx.enter_context(tc.tile_pool(name="lnscratch", bufs=1)),
        "lvec": ctx.enter_context(tc.tile_pool(name="lvec", bufs=2)),
        "lay": ctx.enter_context(tc.tile_pool(name="lay", bufs=1)),
    }

    identity = pers.tile([128, 128], F32, name="identity")
    make_identity(nc, identity[:])
    ones_col = pers.tile([128, 1], F32R, name="ones_col")
    nc.vector.memset(ones_col[:].bitcast(F32), 1.0)
    ones_row = pers.tile([1, 512], F32R, name="ones_row")
    nc.vector.memset(ones_row[:].bitcast(F32), 1.0)
    eps_tile = pers.tile([1, 1], F32, name="eps_tile")
    nc.vector.memset(eps_tile[:], EPS)
    nc._ones_col = ones_col
    nc._ones_row = ones_row
    nc._eps_tile = eps_tile

    xT = pers.tile([128, DC, T], F32R, name="xT")
    hbuf = pers.tile([128, DC, T], F32R, name="hbuf")
    P["spart"] = pers.tile([128, TCN, VTN], F32, name="spart")
    P["ssum"] = pers.tile([128, TCN], F32, name="ssum")

    idx_sb = pers.tile([128, TCN], I32, name="idx_sb")
    nc.sync.dma_start(idx_sb[:], aps["idx"].rearrange("(c p) -> p c", p=128))

    if reps == 1:
        _forward(nc, P, aps, xT, hbuf, idx_sb, identity)
    else:
        with tc.For_i(0, reps, 1):
            _forward(nc, P, aps, xT, hbuf, idx_sb, identity)


_NC_CACHE = {}


def _build(reps=1):
    if reps in _NC_CACHE:
        return _NC_CACHE[reps]
    from contextlib import ExitStack

    nc = bacc.Bacc("TRN2", target_bir_lowering=False, debug=False, num_devices=8)
    aps = {}
    ins = {
        "idx": ([T], I32),
        "tok_emb": ([V, D], F32),
        "pos_emb": ([T, D], F32),
        "ln1_g": ([L, D], F32), "ln1_b": ([L, D], F32),
        "Wq": ([L, H, D, DH], F32R), "Wk": ([L, H, D, DH], F32R),
        "Wv": ([L, H, D, DH], F32R),
        "Wproj": ([L, D, D], F32R), "bproj": ([L, D], F32R),
        "ln2_g": ([L, D], F32), "ln2_b": ([L, D], F32),
        "W1": ([L, D, FF], F32R), "b1": ([L, FF], F32),
        "W2": ([L, FF, D], F32R), "b2": ([L, D], F32R),
        "lnf_g": ([D], F32), "lnf_b": ([D], F32),
        "W_lm": ([D, V], F32R), "b_lm": ([V], F32R),
    }
    for name, (shape, dt) in ins.items():
        aps[name] = nc.dram_tensor(name, shape, dt, kind="ExternalInput").ap()
    aps["logits"] = nc.dram_tensor("logits", [T, V], F32, kind="ExternalOutput").ap()
    aps["sumexp"] = nc.dram_tensor("sumexp", [128, TCN], F32,
                                   kind="ExternalOutput").ap()

    with tile.TileContext(nc) as tc:
        with nc.allow_low_precision(reason="fp32r matmul pipeline"):
            with ExitStack() as ctx:
                _body(ctx, tc, aps, reps=reps)
    nc.compile()
    _NC_CACHE[reps] = nc
    return nc


def kernel(**inputs):
    nc = _build()
    f32 = lambda k: np.ascontiguousarray(np.asarray(inputs[k], dtype=np.float32))
    idx = np.ascontiguousarray(np.asarray(inputs["idx"], dtype=np.int32))
    targets = np.asarray(inputs["targets"])
    shared = {k: f32(k) for k in
              ["tok_emb", "pos_emb", "ln1_g", "ln1_b", "Wq", "Wk", "Wv", "Wproj",
               "bproj", "ln2_g", "ln2_b", "W1", "b1", "W2", "b2", "lnf_g", "lnf_b",
               "W_lm", "b_lm"]}
    in_maps = [dict(shared, idx=idx[b]) for b in range(B)]
    res = bass_utils.run_bass_kernel_spmd(nc, in_maps, core_ids=list(range(B)))

    logits = np.stack([res.results[b]["logits"] for b in range(B)])  # [B, T, V]
    # sumexp comes back [128, TCN] with token t = tc*128 + p at [p, tc]
    s = np.stack([res.results[b]["sumexp"].T.reshape(T) for b in range(B)])  # [B,T]
    tgt = np.take_along_axis(
        logits.astype(np.float64), targets[..., None].astype(np.int64), axis=-1
    )[..., 0]
    loss = np.float32(np.mean(np.log(s.astype(np.float64)) - tgt))
    return logits, loss


# revision 16
# speedup vs baseline: 1.7518x; 1.0505x over previous
"""MiniGPT forward (B=8,T=1024,V=8192,D=768,H=12,L=6) on 8 trn2 NeuronCores.

Sharding: pure data-parallel over batch (1 batch row per core, no collectives).
Per-core layout: activations kept feature-major ("xT": [d partitions, t free]),
which makes every matmul transpose-free:
  - q/k produced directly transposed per head-pair: qT/kT [e, t]
  - v produced token-major [t, e] (augmented with a ones column so the
    attention-probability row-sum falls out of the same matmul)
  - scores computed as scoresT [k, q]; softmax sum over k via the ones row,
    normalization applied to the (tiny) oT output instead of the probs
  - causal mask: tile-level skipping + one 128x128 affine_select per diagonal
    subtile applied after exp (fill=0)
  - LN stats via ones-column matmul reductions; normalize with PE-broadcast
    rows; gains/biases fold into one per-partition scalar-engine activation
  - lm head emits logits token-major plus a fused exp row-sum (accum_out) so
    the cross-entropy reduces on host from sum-exp + gathered target logits.
All matmul operands are float32r (TF32-style) for full PE rate at fp32 I/O.
"""

import numpy as np

import concourse.bass as bass
import concourse.tile as tile
from concourse import bacc, mybir
from concourse import bass_utils
from concourse.masks import make_identity

# Pipelined LDWEIGHTS: walrus is invoked with --enable-ldw-opt=false by
# default; every fused matmul then serializes its weight load. Flip it.
if not getattr(bass_utils, "_ldw_opt_patched", False):
    _orig_run_command = bass_utils.run_command

    def _run_command_ldw(argv, **kwargs):
        argv = ["--enable-ldw-opt=true" if a == "--enable-ldw-opt=false" else a
                for a in argv]
        return _orig_run_command(argv, **kwargs)

    bass_utils.run_command = _run_command_ldw
    bass_utils._ldw_opt_patched = True

B, T, V, D, H, L = 8, 1024, 8192, 768, 12, 6
DH = D // H
FF = 4 * D
EPS = 1e-5
SCALE = float(D) ** -0.5
DC = D // 128          # 6 feature chunks
TTN = T // 512         # 2 t-tiles of 512
TCN = T // 128         # 8 t-chunks of 128
FCN = FF // 128        # 24 ff chunks
FH = FCN // 2          # ff half-block
VTN = V // 512         # 16 vocab tiles
F32 = mybir.dt.float32
F32R = mybir.dt.float32r
I32 = mybir.dt.int32


def _ln_layer(nc, P, xin, gvec, bvec, xout):
    """LayerNorm over the feature (partition) axis of xin [128, DC, T] -> xout."""
    ps, lnvec, lnscratch = P["ps"], P["lnvec"], P["lnscratch"]
    for tt in range(TTN):
        sl = bass.ts(tt, 512)
        s1 = ps.tile([1, 512], F32, tag="mm", bufs=3, name="ln_s1")
        s2 = ps.tile([1, 512], F32, tag="mm", bufs=3, name="ln_s2")
        for dc in range(DC):
            nc.tensor.matmul(s1, nc._ones_col[:], xin[:, dc, sl],
                             start=(dc == 0), stop=(dc == DC - 1))
        for dc in range(DC):
            sq = lnscratch.tile([128, 512], F32R, tag="lnscratch", bufs=3,
                                name="ln_sq")
            nc.scalar.square(sq[:], xin[:, dc, sl])
            nc.tensor.matmul(s2, nc._ones_col[:], sq[:],
                             start=(dc == 0), stop=(dc == DC - 1))
        m = lnvec.tile([1, 512], F32, tag="lnvec", bufs=6, name="ln_m")
        ex2 = lnvec.tile([1, 512], F32, tag="lnvec", bufs=6, name="ln_ex2")
        nc.vector.tensor_scalar_mul(m[:], s1, 1.0 / D)
        nc.vector.tensor_scalar_mul(ex2[:], s2, 1.0 / D)
        var = lnvec.tile([1, 512], F32, tag="lnvec", bufs=6, name="ln_var")
        nc.vector.tensor_mul(var[:], m[:], m[:])
        nc.vector.tensor_tensor(out=var[:], in0=ex2[:], in1=var[:],
                                op=mybir.AluOpType.subtract)
        std = lnvec.tile([1, 512], F32, tag="lnvec", bufs=6, name="ln_std")
        nc.scalar.activation(std[:], var[:], mybir.ActivationFunctionType.Sqrt,
                             bias=nc._eps_tile[:], scale=1.0)
        r = lnvec.tile([1, 512], F32R, tag="lnvec", bufs=6, name="ln_r")
        nc.vector.reciprocal(r[:], std[:])
        mr = lnvec.tile([1, 512], F32R, tag="lnvec", bufs=6, name="ln_mr")
        nc.vector.tensor_mul(mr[:], m[:], r[:])
        rb = ps.tile([128, 512], F32, tag="bc", bufs=2, name="ln_rb")
        nc.tensor.matmul(rb, nc._ones_row[:, :128], r[:], start=True, stop=True)
        mrb = ps.tile([128, 512], F32, tag="bc", bufs=2, name="ln_mrb")
        nc.tensor.matmul(mrb, nc._ones_row[:, :128], mr[:], start=True, stop=True)
        for dc in range(DC):
            u = lnscratch.tile([128, 512], F32, tag="lnscratch", bufs=3,
                               name="ln_u")
            nc.vector.tensor_mul(u[:], xin[:, dc, sl], rb)
            nc.vector.tensor_tensor(out=u[:], in0=u[:], in1=mrb,
                                    op=mybir.AluOpType.subtract)
            nc.scalar.activation(xout[:, dc, sl], u[:],
                                 mybir.ActivationFunctionType.Identity,
                                 bias=bvec[:, dc:dc + 1], scale=gvec[:, dc:dc + 1])


def _forward(nc, P, aps, xT, hbuf, idx_sb, identity):
    """One full forward pass. Pools in P, persistent tiles passed in."""
    ps, lay, lvec = P["ps"], P["lay"], P["lvec"]
    lnvec, lnscratch = P["lnvec"], P["lnscratch"]
    ones_row = nc._ones_row

    # ---- embedding gather + transpose into feature-major xT ----
    for tcn in range(TCN):
        x0 = lnscratch.tile([128, D], F32, tag="lnscratch", bufs=3, name="emb_x0")
        nc.gpsimd.indirect_dma_start(
            out=x0[:], out_offset=None, in_=aps["tok_emb"],
            in_offset=bass.IndirectOffsetOnAxis(ap=idx_sb[:, tcn:tcn + 1], axis=0))
        pos = lnscratch.tile([128, D], F32, tag="lnscratch", bufs=3, name="emb_pos")
        nc.sync.dma_start(pos[:], aps["pos_emb"][bass.ts(tcn, 128), :])
        nc.vector.tensor_add(x0[:], x0[:], pos[:])
        for dc in range(DC):
            tp = ps.tile([128, 128], F32, tag="mm", bufs=3, name="emb_tp")
            nc.tensor.transpose(tp, x0[:, bass.ts(dc, 128)], identity[:])
            nc.vector.tensor_copy(xT[:, dc, bass.ts(tcn, 128)], tp)

    for l in range(L):
        g1 = lvec.tile([128, DC], F32, tag="g1", name="g1")
        nc.sync.dma_start(g1[:], aps["ln1_g"][l].rearrange("(c p) -> p c", p=128))
        b1l = lvec.tile([128, DC], F32, tag="b1l", name="b1l")
        nc.sync.dma_start(b1l[:], aps["ln1_b"][l].rearrange("(c p) -> p c", p=128))
        g2 = lvec.tile([128, DC], F32, tag="g2", name="g2")
        nc.sync.dma_start(g2[:], aps["ln2_g"][l].rearrange("(c p) -> p c", p=128))
        b2l = lvec.tile([128, DC], F32, tag="b2l", name="b2l")
        nc.sync.dma_start(b2l[:], aps["ln2_b"][l].rearrange("(c p) -> p c", p=128))
        bpj = lvec.tile([1, D], F32R, tag="bpj", name="bpj")
        nc.sync.dma_start(bpj[:], aps["bproj"][l:l + 1, :])
        b1f = lvec.tile([128, FCN], F32, tag="b1f", name="b1f")
        nc.sync.dma_start(b1f[:], aps["b1"][l].rearrange("(c p) -> p c", p=128))
        b2f = lvec.tile([1, D], F32R, tag="b2f", name="b2f")
        nc.sync.dma_start(b2f[:], aps["b2"][l:l + 1, :])

        # ---- LN1: xT -> hbuf ----
        _ln_layer(nc, P, xT, g1, b1l, hbuf)

        # ---- qT / kT (feature-major, per head-pair chunks) ----
        qT = lay.tile([128, DC, T], F32R, tag="qT", name="qT")
        kT = lay.tile([128, DC, T], F32R, tag="kT", name="kT")
        for c in range(DC):
            wq = lay.tile([128, DC, 2, DH], F32R, tag="wstream", bufs=2, name="wq")
            wk = lay.tile([128, DC, 2, DH], F32R, tag="wstream", bufs=2, name="wk")
            for hh in range(2):
                nc.sync.dma_start(
                    wq[:, :, hh, :], aps["Wq"][l, 2 * c + hh]
                    .rearrange("(dc p) e -> p dc e", p=128))
                nc.sync.dma_start(
                    wk[:, :, hh, :], aps["Wk"][l, 2 * c + hh]
                    .rearrange("(dc p) e -> p dc e", p=128))
            for tt in range(TTN):
                sl = bass.ts(tt, 512)
                qps = ps.tile([128, 512], F32, tag="mm", bufs=3, name="q_ps")
                for dc in range(DC):
                    nc.tensor.matmul(qps, wq[:, dc].rearrange("p h e -> p (h e)"),
                                     hbuf[:, dc, sl],
                                     start=(dc == 0), stop=(dc == DC - 1))
                nc.scalar.copy(qT[:, c, sl], qps)
                kps = ps.tile([128, 512], F32, tag="mm", bufs=3, name="k_ps")
                for dc in range(DC):
                    nc.tensor.matmul(kps, wk[:, dc].rearrange("p h e -> p (h e)"),
                                     hbuf[:, dc, sl],
                                     start=(dc == 0), stop=(dc == DC - 1))
                nc.scalar.copy(kT[:, c, sl], kps)

        # ---- v (token-major, all heads, augmented with ones column) ----
        v_aug = lay.tile([128, TCN, H, DH + 1], F32R, tag="big", name="v_aug")
        nc.vector.memset(v_aug[:, :, :, DH:DH + 1].bitcast(F32), 1.0)
        wv0 = lay.tile([128, DC, 8, DH], F32R, tag="wstream", bufs=2, name="wv0")
        wv1 = lay.tile([128, DC, 4, DH], F32R, tag="wstream", bufs=2, name="wv1")
        for hh in range(8):
            nc.sync.dma_start(wv0[:, :, hh, :], aps["Wv"][l, hh]
                              .rearrange("(dc p) e -> p dc e", p=128))
        for hh in range(4):
            nc.sync.dma_start(wv1[:, :, hh, :], aps["Wv"][l, 8 + hh]
                              .rearrange("(dc p) e -> p dc e", p=128))
        for tcn in range(TCN):
            v0 = ps.tile([128, 512], F32, tag="mm", bufs=3, name="v0_ps")
            for dc in range(DC):
                nc.tensor.matmul(v0, hbuf[:, dc, bass.ts(tcn, 128)],
                                 wv0[:, dc].rearrange("p h e -> p (h e)"),
                                 start=(dc == 0), stop=(dc == DC - 1))
            nc.vector.tensor_copy(
                v_aug[:, tcn, 0:8, 0:DH], v0.rearrange("p (h e) -> p h e", e=DH))
            v1 = ps.tile([128, 256], F32, tag="mm", bufs=3, name="v1_ps")
            for dc in range(DC):
                nc.tensor.matmul(v1, hbuf[:, dc, bass.ts(tcn, 128)],
                                 wv1[:, dc].rearrange("p h e -> p (h e)"),
                                 start=(dc == 0), stop=(dc == DC - 1))
            nc.vector.tensor_copy(
                v_aug[:, tcn, 8:12, 0:DH], v1.rearrange("p (h e) -> p h e", e=DH))

        # ---- attention (scoresT layout [k, q]) -> hbuf (= concat-head oT) ----
        for h in range(H):
            c, po = h // 2, 64 * (h % 2)
            for qt in range(TTN):
                nk = 4 * (qt + 1)
                ot = ps.tile([65, 512], F32, tag="acc", bufs=3, name="ot_ps")
                for ki in range(nk):
                    rdiag = ki - 4 * qt
                    lo = max(0, 128 * rdiag)   # first valid q column in tile
                    sc = ps.tile([128, 512], F32, tag="mm", bufs=3, name="sc_ps")
                    nc.tensor.matmul(
                        sc[:, lo:], kT[po:po + 64, c, bass.ts(ki, 128)],
                        qT[po:po + 64, c, bass.ds(qt * 512 + lo, 512 - lo)],
                        start=True, stop=True)
                    ex = lay.tile([128, 512], F32R, tag="expT", bufs=3, name="ex")
                    nc.scalar.activation(
                        ex[:, lo:], sc[:, lo:],
                        mybir.ActivationFunctionType.Exp, scale=SCALE)
                    if rdiag >= 0:
                        nc.gpsimd.affine_select(
                            out=ex[:, lo:lo + 128], in_=ex[:, lo:lo + 128],
                            compare_op=mybir.AluOpType.is_ge,
                            fill=0.0, base=0, channel_multiplier=-1,
                            pattern=[[1, 128]])
                    nc.tensor.matmul(
                        ot[:, lo:], v_aug[:, ki, h, :], ex[:, lo:],
                        start=(ki == 0), stop=(ki == nk - 1))
                rr = lnvec.tile([1, 512], F32R, tag="rr", bufs=2, name="rr")
                nc.vector.reciprocal(rr[:], ot[64:65, :])
                rbc = ps.tile([64, 512], F32, tag="bc", bufs=2, name="rbc")
                nc.tensor.matmul(rbc, nc._ones_row[:, :64], rr[:],
                                 start=True, stop=True)
                rbs = lnscratch.tile([64, 512], F32, tag="rbs", bufs=2, name="rbs")
                nc.vector.tensor_copy(rbs[:], rbc)
                nc.vector.tensor_mul(
                    hbuf[po:po + 64, c, bass.ts(qt, 512)], ot[0:64, :], rbs[:])

        # ---- proj + residual into xT ----
        for pc in range(DC):
            wp = lay.tile([128, DC, 128], F32R, tag="wstream", bufs=2, name="wp")
            nc.sync.dma_start(
                wp[:], aps["Wproj"][l].rearrange("(dc p) n -> p dc n", p=128)
                [:, :, bass.ts(pc, 128)])
            for tt in range(TTN):
                sl = bass.ts(tt, 512)
                pj = ps.tile([128, 512], F32, tag="mm", bufs=3, name="pj_ps")
                for dc in range(DC):
                    nc.tensor.matmul(pj, wp[:, dc], hbuf[:, dc, sl],
                                     start=(dc == 0), stop=False)
                nc.tensor.matmul(pj, bpj[:, bass.ts(pc, 128)], ones_row[:],
                                 start=False, stop=True)
                nc.vector.tensor_add(xT[:, pc, sl], xT[:, pc, sl], pj)

        # ---- LN2: xT -> hbuf ----
        _ln_layer(nc, P, xT, g2, b2l, hbuf)

        # ---- FFN + residual into xT (half-ff blocking) ----
        for tt in range(TTN):
            sl = bass.ts(tt, 512)
            o2sb = lay.tile([128, DC, 512], F32, tag="o2sb", name="o2sb")
            for half in range(2):
                hT = lay.tile([128, FH, 512], F32R, tag="big", name="hT")
                for fcl in range(FH):
                    fc = half * FH + fcl
                    w1 = lay.tile([128, DC, 128], F32R, tag="wstream", bufs=2,
                                  name="w1")
                    nc.sync.dma_start(
                        w1[:], aps["W1"][l].rearrange("(dc p) n -> p dc n", p=128)
                        [:, :, bass.ts(fc, 128)])
                    h1 = ps.tile([128, 512], F32, tag="mm", bufs=3, name="h1_ps")
                    for dc in range(DC):
                        nc.tensor.matmul(h1, w1[:, dc], hbuf[:, dc, sl],
                                         start=(dc == 0), stop=(dc == DC - 1))
                    nc.scalar.activation(hT[:, fcl, :], h1,
                                         mybir.ActivationFunctionType.Relu,
                                         bias=b1f[:, fc:fc + 1], scale=1.0)
                for dc2 in range(DC):
                    w2 = lay.tile([128, FH, 128], F32R, tag="wstream", bufs=2,
                                  name="w2")
                    nc.sync.dma_start(
                        w2[:], aps["W2"][l].rearrange("(fc p) n -> p fc n", p=128)
                        [:, bass.ds(half * FH, FH), bass.ts(dc2, 128)])
                    o2 = ps.tile([128, 512], F32, tag="acc", bufs=3, name="o2_ps")
                    for fcl in range(FH):
                        nc.tensor.matmul(o2, w2[:, fcl], hT[:, fcl, :],
                                         start=(fcl == 0),
                                         stop=(half == 0 and fcl == FH - 1))
                    if half == 0:
                        nc.vector.tensor_copy(o2sb[:, dc2, :], o2)
                    else:
                        nc.tensor.matmul(o2, b2f[:, bass.ts(dc2, 128)],
                                         ones_row[:], start=False, stop=True)
                        nc.vector.tensor_add(xT[:, dc2, sl], xT[:, dc2, sl], o2)
                        nc.vector.tensor_add(xT[:, dc2, sl], xT[:, dc2, sl],
                                             o2sb[:, dc2, :])

    # ---- final LN ----
    gf = lvec.tile([128, DC], F32, tag="g1", name="gf")
    nc.sync.dma_start(gf[:], aps["lnf_g"].rearrange("(c p) -> p c", p=128))
    bf = lvec.tile([128, DC], F32, tag="b1l", name="bf")
    nc.sync.dma_start(bf[:], aps["lnf_b"].rearrange("(c p) -> p c", p=128))
    _ln_layer(nc, P, xT, gf, bf, hbuf)

    # ---- lm head + fused sum-exp ----
    spart = P["spart"]
    for vt in range(VTN):
        wlm = lay.tile([128, DC, 512], F32R, tag="wstream", bufs=2, name="wlm")
        nc.sync.dma_start(
            wlm[:], aps["W_lm"].rearrange("(dc p) n -> p dc n", p=128)
            [:, :, bass.ts(vt, 512)])
        blmv = lnvec.tile([1, 512], F32R, tag="rr", bufs=2, name="blmv")
        nc.sync.dma_start(blmv[:], aps["b_lm"].rearrange("(a v) -> a v", a=1)
                          [:, bass.ts(vt, 512)])
        for tcn in range(TCN):
            lg = ps.tile([128, 512], F32, tag="mm", bufs=3, name="lg_ps")
            for dc in range(DC):
                nc.tensor.matmul(lg, hbuf[:, dc, bass.ts(tcn, 128)], wlm[:, dc],
                                 start=(dc == 0), stop=False)
            nc.tensor.matmul(lg, ones_row[:, :128], blmv[:],
                             start=False, stop=True)
            lgs = lay.tile([128, 512], F32, tag="expT", bufs=3, name="lgs")
            nc.vector.tensor_copy(lgs[:], lg)
            nc.sync.dma_start(
                aps["logits"][bass.ts(tcn, 128), bass.ts(vt, 512)], lgs[:])
            exps = lnscratch.tile([128, 512], F32, tag="lnscratch", bufs=3,
                                  name="exps")
            nc.scalar.activation(exps[:], lgs[:],
                                 mybir.ActivationFunctionType.Exp,
                                 accum_out=spart[:, tcn, vt:vt + 1])
    ssum = P["ssum"]
    for tcn in range(TCN):
        nc.vector.reduce_sum(out=ssum[:, tcn:tcn + 1], in_=spart[:, tcn, :],
                             axis=mybir.AxisListType.X)
    nc.sync.dma_start(aps["sumexp"], ssum[:])


def _body(ctx, tc, aps, reps=1):
    nc = tc.nc
    pers = ctx.enter_context(tc.tile_pool(name="pers", bufs=1))
    P = {
        "ps": ctx.enter_context(tc.tile_pool(name="ps", bufs=1, space="PSUM")),
        "lnvec": ctx.enter_context(tc.tile_pool(name="lnvec", bufs=1)),
        "lnscratch": ctx.enter_context(tc.tile_pool(name="lnscratch", bufs=1)),
        "lvec": ctx.enter_context(tc.tile_pool(name="lvec", bufs=2)),
        "lay": ctx.enter_context(tc.tile_pool(name="lay", bufs=1)),
    }

    identity = pers.tile([128, 128], F32, name="identity")
    make_identity(nc, identity[:])
    ones_col = pers.tile([128, 1], F32R, name="ones_col")
    nc.vector.memset(ones_col[:].bitcast(F32), 1.0)
    ones_row = pers.tile([1, 512], F32R, name="ones_row")
    nc.vector.memset(ones_row[:].bitcast(F32), 1.0)
    eps_tile = pers.tile([1, 1], F32, name="eps_tile")
    nc.vector.memset(eps_tile[:], EPS)
    nc._ones_col = ones_col
    nc._ones_row = ones_row
    nc._eps_tile = eps_tile

    xT = pers.tile([128, DC, T], F32R, name="xT")
    hbuf = pers.tile([128, DC, T], F32R, name="hbuf")
    P["spart"] = pers.tile([128, TCN, VTN], F32, name="spart")
    P["ssum"] = pers.tile([128, TCN], F32, name="ssum")

    idx_sb = pers.tile([128, TCN], I32, name="idx_sb")
    nc.sync.dma_start(idx_sb[:], aps["idx"].rearrange("(c p) -> p c", p=128))

    if reps == 1:
        _forward(nc, P, aps, xT, hbuf, idx_sb, identity)
    else:
        with tc.For_i(0, reps, 1):
            _forward(nc, P, aps, xT, hbuf, idx_sb, identity)


_NC_CACHE = {}


def _build(reps=1):
    if reps in _NC_CACHE:
        return _NC_CACHE[reps]
    from contextlib import ExitStack

    nc = bacc.Bacc("TRN2", target_bir_lowering=False, debug=False, num_devices=8)
    aps = {}
    ins = {
        "idx": ([T], I32),
        "tok_emb": ([V, D], F32),
        "pos_emb": ([T, D], F32),
        "ln1_g": ([L, D], F32), "ln1_b": ([L, D], F32),
        "Wq": ([L, H, D, DH], F32R), "Wk": ([L, H, D, DH], F32R),
        "Wv": ([L, H, D, DH], F32R),
        "Wproj": ([L, D, D], F32R), "bproj": ([L, D], F32R),
        "ln2_g": ([L, D], F32), "ln2_b": ([L, D], F32),
        "W1": ([L, D, FF], F32R), "b1": ([L, FF], F32),
        "W2": ([L, FF, D], F32R), "b2": ([L, D], F32R),
        "lnf_g": ([D], F32), "lnf_b": ([D], F32),
        "W_lm": ([D, V], F32R), "b_lm": ([V], F32R),
    }
    for name, (shape, dt) in ins.items():
        aps[name] = nc.dram_tensor(name, shape, dt, kind="ExternalInput").ap()
    aps["logits"] = nc.dram_tensor("logits", [T, V], F32, kind="ExternalOutput").ap()
    aps["sumexp"] = nc.dram_tensor("sumexp", [128, TCN], F32,
                                   kind="ExternalOutput").ap()

    with tile.TileContext(nc) as tc:
        with nc.allow_low_precision(reason="fp32r matmul pipeline"):
            with ExitStack() as ctx:
                _body(ctx, tc, aps, reps=reps)
    nc.compile()
    _NC_CACHE[reps] = nc
    return nc


def kernel(**inputs):
    nc = _build()
    f32 = lambda k: np.ascontiguousarray(np.asarray(inputs[k], dtype=np.float32))
    idx = np.ascontiguousarray(np.asarray(inputs["idx"], dtype=np.int32))
    targets = np.asarray(inputs["targets"])
    shared = {k: f32(k) for k in
              ["tok_emb", "pos_emb", "ln1_g", "ln1_b", "Wq", "Wk", "Wv", "Wproj",
               "bproj", "ln2_g", "ln2_b", "W1", "b1", "W2", "b2", "lnf_g", "lnf_b",
               "W_lm", "b_lm"]}
    in_maps = [dict(shared, idx=idx[b]) for b in range(B)]
    res = bass_utils.run_bass_kernel_spmd(nc, in_maps, core_ids=list(range(B)))

    logits = np.stack([res.results[b]["logits"] for b in range(B)])  # [B, T, V]
    # sumexp comes back [128, TCN] with token t = tc*128 + p at [p, tc]
    s = np.stack([res.results[b]["sumexp"].T.reshape(T) for b in range(B)])  # [B,T]
    tgt = np.take_along_axis(
        logits.astype(np.float64), targets[..., None].astype(np.int64), axis=-1
    )[..., 0]
    loss = np.float32(np.mean(np.log(s.astype(np.float64)) - tgt))
    return logits, loss
